# revision 4
# baseline (speedup 1.0000x reference)
"""Trainium2 Bass kernel for 2-layer GAT (nn_GAT_22634477650567).

8 NeuronCores, tensor-parallel over the H=8 heads: each core computes one
head per layer.  All large tensors stay "T-major" ([feature, node]) so the
attention matmul needs no transposes:

  - scores built k-major: p^T[k, q] = mask[q,k] * exp(lrelu(src_q + dst_k))
  - ACT recipe: Prelu(src_bcast + dst_k, alpha=0.2) then Exp (scalar engine)
  - DVE recipe: p^T = max(e^src * e^dst_k, e^.2src * e^.2dst_k) via a custom
    fused DVE op, using exp(lrelu(a+b)) == max(e^a e^b, e^.2a e^.2b)
  - out^T[o, q] = sum_k p^T[k,q] * WhAug[k, o] accumulated in PSUM; a ones
    column in WhAug yields the softmax denominator for free.

Per-layer head outputs are AllGathered ([32,4096] each -> [256,4096]), the
ELU+residual applied locally, and layer 2 repeats.  The host transposes and
concats the per-core [32, 4096] outputs.

bf16 is used only where rounding cancels in the softmax (per-query factors)
or where it matches the matmul operand precision; dst-side quantities stay
fp32.
"""

import numpy as np
import ml_dtypes

import concourse.bass as bass
import concourse.mybir as mybir
import concourse.tile as tile
from concourse import bacc
from concourse.bass_utils import run_bass_kernel_spmd

# ---------------- custom DVE op: out = max(in0*s0, in1*s1) ----------------
import concourse.dve_ops as dve_ops
from concourse.dve_spec import (
    Src0,
    Src1,
    C0,
    C1,
    maxx,
    lower as dve_lower,
    Spec as DveSpec,
)
from concourse.dve_uop import DveOpSpec


def _register_maxprod():
    name = "MAXPROD_ANT"
    for op in dve_ops.OPS:
        if op.name == name:
            return op
    spec = DveSpec(
        body=maxx(Src0 * C0, Src1 * C1),
        reference=lambda in0, in1, s0, s1, imm2: np.maximum(in0 * s0, in1 * s1).astype(
            np.float32
        ),
    )
    opcode = dve_ops._CUSTOM_DVE_ROW_BASE + len(dve_ops.OPS)
    shas = {}
    for ver in ("v3", "v4"):
        s = DveOpSpec(
            name=name, opcode=opcode, uops=dve_lower(spec, ver=ver), rd1_en=True
        )
        shas[ver] = s.sha(ver)
    op = dve_ops.DveOp(name, spec, subdim=False, uops_sha=shas)
    dve_ops.OPS.append(op)
    dve_ops.CUSTOM_DVE_SPECS[name] = spec
    dve_ops._SUB_OPCODE_FOR_NAME[name] = opcode
    return op


MAXPROD = _register_maxprod()

F32 = mybir.dt.float32
BF16 = mybir.dt.bfloat16
AF = mybir.ActivationFunctionType
ALU = mybir.AluOpType

N = 4096          # nodes
D = 256           # input features
O = 32            # per-head output features
P = 128           # partitions
NCH = N // P      # 32 k-chunks
NB = N // 512     # 8 psum bank columns
NCORE = 8
LRELU = 0.2

# which k-chunks use the DVE (custom-op) recipe vs the ACT (Prelu+Exp)
# recipe; ~7/16 on DVE balances ACT vs DVE engine time.
DVE_CHUNKS = frozenset(c for c in range(NCH) if c % 16 in (1, 3, 5, 7, 9, 11, 13))
MASK_DMA_SPLIT = 2  # sub-DMAs per 1MB mask chunk

# column offsets inside the packed "smalls" f32 tile [128, 1024]
SM_WT0, SM_WT1 = 0, 32        # W chunks [128, 32] each
SM_DCOL, SM_VCOL, SM_BCOL = 64, 96, 128
SM_ONES, SM_ONES33, SM_AT = 160, 288, 320


def _elu_plus_into(nc, out_ap, in_ap, res_ap, t1_ap, t2_ap):
    """out = elu(in) + res  (out may alias res; t1/t2 are scratch).

    elu(x) = relu(x) + exp(min(x, 0)) - 1
    """
    nc.vector.tensor_scalar_min(t1_ap, in_ap, 0.0)
    nc.scalar.activation(t2_ap, t1_ap, AF.Exp)
    nc.vector.scalar_tensor_tensor(t1_ap, in_ap, 0.0, t2_ap, ALU.max, ALU.add)
    nc.vector.scalar_tensor_tensor(out_ap, t1_ap, -1.0, res_ap, ALU.add, ALU.add)


def _gat_layer(nc, tc, pools, layer, xt_tiles, w_dram, a_dram, mask_dram):
    """One GAT head layer. Returns the normalized head output tile
    ([32, 4096] f32, T-major), allocated from the big pool (tag 'ubc')."""
    sb = pools["sb"]
    big = pools["big"]
    L = layer

    sm = sb.tile([P, 1024], F32, name=f"smalls{L}", tag="smalls")
    nc.sync.dma_start(sm[:, SM_WT0:SM_WT0 + O], w_dram[0:P, :])
    nc.sync.dma_start(sm[:, SM_WT1:SM_WT1 + O], w_dram[P:D, :])
    nc.sync.dma_start(sm[0:O, SM_AT:SM_AT + 2], a_dram[:])
    nc.vector.memset(sm[0:1, SM_ONES:SM_ONES + P], 1.0)
    onesb = sb.tile([1, P], BF16, name=f"onesb{L}", tag="onesb")
    nc.vector.memset(onesb[:], 1.0)
    nc.vector.memset(sm[0:33, SM_ONES33:SM_ONES33 + O], 1.0)

    whaug = sb.tile([P, 33 * NCH], BF16, name=f"whaug{L}", tag="whaug")
    nc.vector.memset(whaug[:], 1.0)
    whT = sb.tile([O, N], F32, name=f"whT{L}", tag="whT")
    sbc = big.tile([P, N], BF16, name=f"sbc{L}", tag="sbc")

    with tc.tile_pool(name=f"sps{L}", bufs=3, space="PSUM") as sps:
        # Wh [n, o] chunks -> whaug (bf16) with ones column
        for c in range(NCH):
            pw = sps.tile([P, O], F32, name=f"pw{L}_{c}", tag="ps")
            for dc in range(2):
                nc.tensor.matmul(
                    pw[:],
                    xt_tiles[dc][:, c * P:(c + 1) * P],
                    sm[:, SM_WT0 + dc * O: SM_WT0 + dc * O + O],
                    start=(dc == 0),
                    stop=(dc == 1),
                )
            nc.any.tensor_copy(whaug[:, c * 33: c * 33 + 32], pw[:])

        # WhT [o, n] (f32)
        for g in range(NB):
            pt = sps.tile([O, 512], F32, name=f"pt{L}_{g}", tag="ps")
            for dc in range(2):
                nc.tensor.matmul(
                    pt[:],
                    sm[:, SM_WT0 + dc * O: SM_WT0 + dc * O + O],
                    xt_tiles[dc][:, g * 512:(g + 1) * 512],
                    start=(dc == 0),
                    stop=(dc == 1),
                )
            nc.any.tensor_copy(whT[:, g * 512:(g + 1) * 512], pt[:])

        # src row (bf16, stored in sbc row 0)
        for g in range(NB):
            pr = sps.tile([1, 512], F32, name=f"pr{L}_{g}", tag="ps")
            nc.tensor.matmul(
                pr[:], sm[0:O, SM_AT:SM_AT + 1],
                whT[:, g * 512:(g + 1) * 512], start=True, stop=True,
            )
            nc.any.tensor_copy(sbc[0:1, g * 512:(g + 1) * 512], pr[:])

        # dst col [128, 32] f32
        dps = sps.tile([P, NCH], F32, name=f"dps{L}", tag="ps")
        for c in range(NCH):
            nc.tensor.matmul(
                dps[:, c:c + 1], whT[:, c * P:(c + 1) * P],
                sm[0:O, SM_AT + 1:SM_AT + 2], start=True, stop=True,
            )
        nc.any.tensor_copy(sm[:, SM_DCOL:SM_DCOL + NCH], dps[:])
        nc.scalar.activation(sm[:, SM_VCOL:SM_VCOL + NCH],
                             sm[:, SM_DCOL:SM_DCOL + NCH], AF.Exp)
        nc.scalar.activation(sm[:, SM_BCOL:SM_BCOL + NCH],
                             sm[:, SM_DCOL:SM_DCOL + NCH], AF.Exp, scale=LRELU)

        # src broadcast [128, 4096] (bf16) from its own row 0
        for g in range(NB):
            pb = sps.tile([P, 512], F32, name=f"pb{L}_{g}", tag="ps")
            nc.tensor.matmul(
                pb[:], onesb[:],
                sbc[0:1, g * 512:(g + 1) * 512], start=True, stop=True,
            )
            nc.any.tensor_copy(sbc[:, g * 512:(g + 1) * 512], pb[:])

    ubc = big.tile([P, N], BF16, name=f"ubc{L}", tag="ubc")
    nc.scalar.activation(ubc[:], sbc[:], AF.Exp)
    abc = big.tile([P, N], BF16, name=f"abc{L}", tag="abc")
    nc.scalar.activation(abc[:], sbc[:], AF.Exp, scale=LRELU)

    # ---------------- hot loop ----------------
    with tc.tile_pool(name=f"aps{L}", bufs=1, space="PSUM") as aps:
        acc = aps.tile([33, N], F32, name=f"acc{L}")
        for c in range(NCH):
            mk = pools["mask"].tile([P, N], BF16, name=f"mk{L}_{c}", tag="mk")
            rows = P // MASK_DMA_SPLIT
            for s in range(MASK_DMA_SPLIT):
                nc.sync.dma_start(
                    mk[s * rows:(s + 1) * rows, :],
                    mask_dram[c * P + s * rows: c * P + (s + 1) * rows, :],
                )
            pp = pools["pp"].tile([P, N], BF16, name=f"pp{L}_{c}", tag="pp")
            if c in DVE_CHUNKS:
                nc.vector._custom_dve(
                    MAXPROD, out=pp[:], in0=ubc[:], in1=abc[:],
                    s0=sm[:, SM_VCOL + c:SM_VCOL + c + 1],
                    s1=sm[:, SM_BCOL + c:SM_BCOL + c + 1],
                )
            else:
                yy = pools["yy"].tile([P, N], F32, name=f"yy{L}_{c}", tag="yy")
                nc.scalar.activation(yy[:], sbc[:], AF.Prelu,
                                     bias=sm[:, SM_DCOL + c:SM_DCOL + c + 1],
                                     alpha=LRELU)
                nc.scalar.activation(pp[:], yy[:], AF.Exp)
            # in-place mask multiply
            nc.vector.tensor_mul(pp[:], pp[:], mk[:])
            for g in range(NB):
                nc.tensor.matmul(
                    acc[:, g * 512:(g + 1) * 512],
                    whaug[:, c * 33: c * 33 + 33],
                    pp[:, g * 512:(g + 1) * 512],
                    start=(c == 0),
                    stop=(c == NCH - 1),
                )

        # pull numerators + reciprocal of denominators out of PSUM
        oT = big.tile([33, N], F32, name=f"oT{L}", tag="sbc")
        nc.vector.reciprocal(oT[32:33, :], acc[32:33, :])
        nc.any.tensor_copy(oT[0:32, :], acc[0:32, :])

    on = big.tile([O, N], F32, name=f"on{L}", tag="ubc")
    with tc.tile_pool(name=f"rps{L}", bufs=1, space="PSUM") as rps:
        rb = rps.tile([O, N], F32, name=f"rb{L}")
        for g in range(NB):
            nc.tensor.matmul(
                rb[:, g * 512:(g + 1) * 512],
                sm[32:33, SM_ONES33:SM_ONES33 + O],
                oT[32:33, g * 512:(g + 1) * 512], start=True, stop=True,
            )
        nc.vector.tensor_mul(on[:], oT[0:32, :], rb[:])
    return on


def build_kernel():
    nc = bacc.Bacc("TRN2", target_bir_lowering=False, debug=False,
                   num_devices=NCORE)

    xT_d = nc.dram_tensor("xT", [D, N], F32, kind="ExternalInput")
    xTown_d = nc.dram_tensor("xTown", [O, N], F32, kind="ExternalInput")
    w1_d = nc.dram_tensor("w1", [D, O], F32, kind="ExternalInput")
    w2_d = nc.dram_tensor("w2", [D, O], F32, kind="ExternalInput")
    a1_d = nc.dram_tensor("a1", [O, 2], F32, kind="ExternalInput")
    a2_d = nc.dram_tensor("a2", [O, 2], F32, kind="ExternalInput")
    mask_d = nc.dram_tensor("maskT", [N, N], BF16, kind="ExternalInput")
    outT_d = nc.dram_tensor("outT", [O, N], F32, kind="ExternalOutput")

    with tile.TileContext(nc) as tc:
        with (
            tc.tile_pool(name="sb", bufs=1) as sb,
            tc.tile_pool(name="big", bufs=1) as big,
            tc.tile_pool(name="mask", bufs=2) as mask_pool,
            tc.tile_pool(name="pp", bufs=2) as pp_pool,
            tc.tile_pool(name="yy", bufs=1) as yy_pool,
            tc.tile_pool(name="dram", bufs=1, space="DRAM") as dram,
        ):
            pools = dict(sb=sb, big=big, mask=mask_pool, pp=pp_pool, yy=yy_pool)

            xt0 = big.tile([P, N], F32, name="xt0", tag="hx0")
            nc.sync.dma_start(xt0[:], xT_d[0:P, :])
            xt1 = big.tile([P, N], F32, name="xt1", tag="hx1")
            nc.sync.dma_start(xt1[:], xT_d[P:D, :])

            # ---- layer 1 ----
            o1n = _gat_layer(nc, tc, pools, 1, (xt0, xt1), w1_d, a1_d, mask_d)

            # gather all heads' outputs
            gin = dram.tile([O, N], F32, name="gin")
            nc.sync.dma_start(gin[:], o1n[:])
            catT = dram.tile([D, N], F32, name="catT", addr_space="Shared")
            nc.gpsimd.collective_compute(
                "AllGather", ALU.bypass,
                replica_groups=[list(range(NCORE))],
                ins=[gin.opt()], outs=[catT.opt()],
            )

            # own-rows residual: hown = elu(o1n) + xTown, spilled to DRAM
            s1 = sb.tile([O, N], F32, name="xown", tag="s1")
            nc.sync.dma_start(s1[:], xTown_d[:])
            s2 = sb.tile([P, N], F32, name="s2a", tag="s2")
            yyt = yy_pool.tile([P, N], F32, name="yyh", tag="yy")
            _elu_plus_into(nc, s2[0:O, :], o1n[:], s1[:], yyt[0:O, :], s2[0:O, :])
            hown_d = dram.tile([O, N], F32, name="hown_d")
            nc.sync.dma_start(hown_d[:], s2[0:O, :])

            # h^T = elu(catT) + xT, in place over xt tiles
            for half, xt in ((0, xt0), (1, xt1)):
                ct = big.tile([P, N], F32, name=f"ct{half}", tag="abc")
                nc.sync.dma_start(ct[:], catT[half * P:(half + 1) * P, :])
                s2b = sb.tile([P, N], F32, name=f"s2b{half}", tag="s2")
                yyb = yy_pool.tile([P, N], F32, name=f"yyb{half}", tag="yy")
                _elu_plus_into(nc, xt[:], ct[:], xt[:], yyb[:], s2b[:])

            # ---- layer 2 ----
            o2n = _gat_layer(nc, tc, pools, 2, (xt0, xt1), w2_d, a2_d, mask_d)

            # out = elu(o2n) + hown
            hr = sb.tile([O, N], F32, name="hr", tag="s1")
            nc.sync.dma_start(hr[:], hown_d[:])
            s2c = sb.tile([P, N], F32, name="s2c", tag="s2")
            yyf = yy_pool.tile([P, N], F32, name="yyf", tag="yy")
            _elu_plus_into(nc, hr[:], o2n[:], hr[:], yyf[0:O, :], s2c[0:O, :])
            nc.sync.dma_start(outT_d[:], hr[:])

    nc.compile()
    return nc


_NC_CACHE = None


def _get_nc():
    global _NC_CACHE
    if _NC_CACHE is None:
        _NC_CACHE = build_kernel()
    return _NC_CACHE


def kernel(x, adj_mat, W1, a1, W2, a2, _trace=False, _tmpdir=None):
    x = np.asarray(x, dtype=np.float32)
    adj = np.asarray(adj_mat)
    W1 = np.asarray(W1, dtype=np.float32)
    a1 = np.asarray(a1, dtype=np.float32)
    W2 = np.asarray(W2, dtype=np.float32)
    a2 = np.asarray(a2, dtype=np.float32)

    xT = np.ascontiguousarray(x.T)                       # [256, 4096]
    maskT = np.ascontiguousarray(adj.T > 0).astype(ml_dtypes.bfloat16)

    nc = _get_nc()
    in_maps = []
    for j in range(NCORE):
        in_maps.append(
            dict(
                xT=xT,
                xTown=np.ascontiguousarray(xT[j * O:(j + 1) * O]),
                w1=np.ascontiguousarray(W1[j]),
                w2=np.ascontiguousarray(W2[j]),
                a1=np.ascontiguousarray(np.stack([a1[j, :O], a1[j, O:]], axis=1)),
                a2=np.ascontiguousarray(np.stack([a2[j, :O], a2[j, O:]], axis=1)),
                maskT=maskT,
            )
        )
    kw = {}
    if _trace:
        kw = dict(trace=True, tmpdir=_tmpdir)
    res = run_bass_kernel_spmd(nc, in_maps, list(range(NCORE)), **kw)
    out = np.empty((N, NCORE * O), dtype=np.float32)
    for j in range(NCORE):
        out[:, j * O:(j + 1) * O] = res.results[j]["outT"].T
    if _trace:
        return out, res
    return out
